# revision 2
# baseline (speedup 1.0000x reference)
"""Llama GQA attention (B=4,S=1024,H=4096,NH=32,NKV=8,D=128) on 8 TRN2 cores.

Strategy: tensor-parallel over heads (4 q heads + 1 kv head per core).
Each core computes qkv-proj + RoPE + causal attention + its partial o_proj
(row slice of w_o); partials are summed on host (the "all-reduce").

Everything on-device flows in transposed [feature, token] layout so no
activation transposes are needed:
  qkv^T = w_qkv_slice.T @ hidden^T      (fp32r matmuls, K=H)
  scores^T[sk,sq] = (k^T slice).T @ q^T (K=D)
  probs^T = exp(scale*scores^T) * causal_mask ; denominator via ones-matmul
  attn^T[d,sq] = v_nat.T @ probs^T      (v transposed on PE, K=sk)
  out^T = w_o_slice.T @ attn^T          (bf16, K=local hid)
Host transposes hidden once, slices weights per core, builds cos/sin tables
from positions, and sums/transposes the per-core partial outputs.
"""

import numpy as np
import ml_dtypes

B, S, H = 4, 1024, 4096
NH, NKV, D = 32, 8, 128
THETA = 10000.0
N_CORES = 8
NHL = NH // N_CORES            # 4 local q heads
TOK = B * S                    # 4096 tokens
NT = TOK // 512                # 8 token tiles
KT = H // 128                  # 32 contraction tiles for qkv
QKV_COLS = (NHL + 2) * D       # 768 local qkv columns
WO_K = NHL * D                 # 512 local o_proj contraction
SCALE = 1.0 / float(np.sqrt(D))

_PROG = {}


def _build_program():
    import concourse.mybir as mybir
    import concourse.tile as tile
    from concourse import bacc
    from concourse.masks import make_identity

    F32 = mybir.dt.float32
    F32R = mybir.dt.float32r
    BF16 = mybir.dt.bfloat16
    MUL = mybir.AluOpType.mult
    ADD = mybir.AluOpType.add
    SUB = mybir.AluOpType.subtract
    EXP = mybir.ActivationFunctionType.Exp

    nc = bacc.Bacc("TRN2", target_bir_lowering=False, debug=False,
                   num_devices=N_CORES)

    hT_d = nc.dram_tensor("hT", (H, TOK), F32R, kind="ExternalInput")
    wq_d = nc.dram_tensor("wq", (H, QKV_COLS), F32R, kind="ExternalInput")
    wo_d = nc.dram_tensor("wo", (WO_K, H), BF16, kind="ExternalInput")
    cs_d = nc.dram_tensor("cs", (64, S), F32, kind="ExternalInput")
    sn_d = nc.dram_tensor("sn", (64, S), F32, kind="ExternalInput")
    mk_d = nc.dram_tensor("mk", (4, 128, 512), BF16, kind="ExternalInput")
    on_d = nc.dram_tensor("on", (128, 128), F32R, kind="ExternalInput")
    outT_d = nc.dram_tensor("outT", (H, TOK), F32, kind="ExternalOutput")

    with nc.allow_low_precision(reason="fp32r operand rounding"), \
         tile.TileContext(nc) as tc:
        with tc.tile_pool(name="persist", bufs=1) as pp:
            attnT = pp.tile([128, NHL, TOK], BF16)
            cs_t = pp.tile([64, S], F32)
            nc.sync.dma_start(cs_t[:], cs_d[:])
            sn_t = pp.tile([64, S], F32)
            nc.sync.dma_start(sn_t[:], sn_d[:])
            mk_t = pp.tile([128, 4, 512], BF16)
            nc.sync.dma_start(mk_t[:], mk_d.rearrange("m p f -> p m f"))
            on_t = pp.tile([128, 128], F32R)
            nc.sync.dma_start(on_t[:], on_d[:])
            ident = pp.tile([128, 128], F32)
            make_identity(nc, ident[:])

            # ---------------- Phase A: qkv + rope + attention ----------------
            with (
                tc.tile_pool(name="pa1", bufs=1) as pa1,
                tc.tile_pool(name="pa2", bufs=2) as pa2,
                tc.tile_pool(name="pa3", bufs=3) as pa3,
                tc.tile_pool(name="paps", bufs=1, space="PSUM") as paps,
            ):
                wq_sb = pa1.tile([128, KT, QKV_COLS], F32R, tag="wq")
                wq_r = wq_d.rearrange("(ko ki) c -> ki ko c", ki=128)
                for k in range(KT):
                    nc.sync.dma_start(wq_sb[:, k, :], wq_r[:, k, :])

                kT_t = None
                v_t = None
                for n in range(NT):
                    b, half = n // 2, n % 2
                    csl = cs_t[:, half * 512:(half + 1) * 512]
                    snl = sn_t[:, half * 512:(half + 1) * 512]

                    # qkv^T for this token tile: 6 col-tiles, k-outer
                    psm = [paps.tile([128, 512], F32, tag=f"qm{m}", name=f"psm{m}")
                           for m in range(6)]
                    for k in range(KT):
                        hT_t = pa2.tile([128, 512], F32R, tag="hT")
                        nc.sync.dma_start(
                            hT_t[:],
                            hT_d[k * 128:(k + 1) * 128, n * 512:(n + 1) * 512])
                        for m in range(6):
                            nc.tensor.matmul(
                                psm[m][:], wq_sb[:, k, m * 128:(m + 1) * 128],
                                hT_t[:], start=(k == 0), stop=(k == KT - 1))

                    # RoPE epilogues (q heads m=0..3, k m=4)
                    qT_t = pa2.tile([128, NHL, 512], F32R, tag="qT")
                    if half == 0:
                        kT_t = pa2.tile([128, S], F32R, tag="kT")
                        v_t = pa2.tile([128, 8, 128], F32R, tag="v")
                    for m in range(5):
                        ps = psm[m]
                        if m < NHL:
                            o1 = qT_t[0:64, m, :]
                            o2 = qT_t[64:128, m, :]
                        else:
                            o1 = kT_t[0:64, half * 512:(half + 1) * 512]
                            o2 = kT_t[64:128, half * 512:(half + 1) * 512]
                        t1 = pa2.tile([64, 512], F32, tag="t1")
                        t2 = pa2.tile([64, 512], F32, tag="t2")
                        nc.vector.tensor_tensor(t1[:], ps[0:64, :], csl, op=MUL)
                        nc.vector.tensor_tensor(t2[:], ps[64:128, :], snl, op=MUL)
                        nc.vector.tensor_tensor(o1, t1[:], t2[:], op=SUB)
                        t3 = pa2.tile([64, 512], F32, tag="t1")
                        t4 = pa2.tile([64, 512], F32, tag="t2")
                        nc.vector.tensor_tensor(t3[:], ps[64:128, :], csl, op=MUL)
                        nc.vector.tensor_tensor(t4[:], ps[0:64, :], snl, op=MUL)
                        nc.vector.tensor_tensor(o2, t3[:], t4[:], op=ADD)

                    # v: evict then transpose to natural [token, d] layout
                    vT_tmp = pa1.tile([128, 512], F32, tag="vT")
                    nc.scalar.copy(vT_tmp[:], psm[5][:])
                    for c4 in range(4):
                        pt = paps.tile([128, 128], F32, tag="qm5")
                        nc.tensor.transpose(
                            pt[:], vT_tmp[:, c4 * 128:(c4 + 1) * 128], ident[:])
                        nc.vector.tensor_copy(v_t[:, half * 4 + c4, :], pt[:])

                    # causal attention for this (batch, sq-tile)
                    jmax = 4 + half * 4
                    for h in range(NHL):
                        ps_attn = paps.tile([128, 512], F32, tag="attn")
                        acc = pa1.tile([128, 512], F32R, tag="acc")
                        for j in range(jmax):
                            ps_s = paps.tile([128, 512], F32, tag="ps_s")
                            nc.tensor.matmul(
                                ps_s[:], kT_t[:, j * 128:(j + 1) * 128],
                                qT_t[:, h, :], start=True, stop=True)
                            probs = pa3.tile([128, 512], F32R, tag="probs")
                            nc.scalar.activation(probs[:], ps_s[:], EXP,
                                                 scale=SCALE)
                            mi = j - half * 4
                            if mi >= 0:
                                nc.vector.tensor_tensor(
                                    probs[:], probs[:], mk_t[:, mi, :], op=MUL)
                            nc.tensor.matmul(
                                ps_attn[:], v_t[:, j, :], probs[:],
                                start=(j == 0), stop=(j == jmax - 1))
                            if j == 0:
                                nc.vector.tensor_copy(acc[:], probs[:])
                            else:
                                nc.vector.tensor_tensor(acc[:], acc[:],
                                                        probs[:], op=ADD)
                        den = paps.tile([1, 512], F32, tag="ps_s")
                        nc.tensor.matmul(den[:], on_t[:, 0:1], acc[:],
                                         start=True, stop=True)
                        recip = pa2.tile([1, 512], F32R, tag="recip")
                        nc.vector.reciprocal(recip[:], den[:])
                        bc_ps = paps.tile([128, 512], F32, tag="ps_s")
                        nc.tensor.matmul(bc_ps[:], on_t[0:1, :], recip[:],
                                         start=True, stop=True)
                        bc_sb = pa1.tile([128, 512], F32, tag="bc")
                        nc.scalar.copy(bc_sb[:], bc_ps[:])
                        nc.vector.tensor_tensor(
                            attnT[:, h, n * 512:(n + 1) * 512],
                            ps_attn[:], bc_sb[:], op=MUL)

            # ---------------- Phase B: o_proj ----------------
            with (
                tc.tile_pool(name="pb3", bufs=3) as pb3,
                tc.tile_pool(name="pbps", bufs=2, space="PSUM") as pbps,
            ):
                wo_r = wo_d.rearrange("(kb ki) m -> ki kb m", ki=128)
                for m in range(H // 128):
                    wo_t = pb3.tile([128, 4, 128], BF16, tag="wo")
                    nc.sync.dma_start(wo_t[:],
                                      wo_r[:, :, m * 128:(m + 1) * 128])
                    for n2 in range(NT):
                        po = pbps.tile([128, 512], F32, tag="po")
                        for kb in range(4):
                            nc.tensor.matmul(
                                po[:], wo_t[:, kb, :],
                                attnT[:, kb, n2 * 512:(n2 + 1) * 512],
                                start=(kb == 0), stop=(kb == 3))
                        ob = pb3.tile([128, 512], F32, tag="ob")
                        nc.scalar.copy(ob[:], po[:])
                        nc.sync.dma_start(
                            outT_d[m * 128:(m + 1) * 128,
                                   n2 * 512:(n2 + 1) * 512], ob[:])

    nc.compile()
    return nc


def _get_program():
    if "nc" not in _PROG:
        _PROG["nc"] = _build_program()
    return _PROG["nc"]


def _host_inputs(positions, hidden_states, w_qkv, w_o):
    positions = np.asarray(positions)
    hidden_states = np.asarray(hidden_states, dtype=np.float32)
    w_qkv = np.asarray(w_qkv, dtype=np.float32)
    w_o = np.asarray(w_o, dtype=np.float32)

    hT = np.ascontiguousarray(hidden_states.reshape(TOK, H).T)

    pos0 = positions[0].astype(np.float32)
    inv = 1.0 / (THETA ** (np.arange(64, dtype=np.float32) / 64.0))
    ang = inv[:, None] * pos0[None, :]            # [64, S]
    cs = np.cos(ang).astype(np.float32)
    sn = np.sin(ang).astype(np.float32)

    mk = np.zeros((4, 128, 512), dtype=ml_dtypes.bfloat16)
    p = np.arange(128)[:, None]
    f = np.arange(512)[None, :]
    for mi in range(4):
        mk[mi] = (p + 128 * mi <= f).astype(ml_dtypes.bfloat16)
    ones = np.ones((128, 128), dtype=np.float32)

    in_maps = []
    for c in range(N_CORES):
        q0 = c * NHL * D
        kc = NH * D + c * D
        vc = NH * D + NKV * D + c * D
        wq = np.ascontiguousarray(np.concatenate(
            [w_qkv[:, q0:q0 + NHL * D],
             w_qkv[:, kc:kc + D],
             w_qkv[:, vc:vc + D]], axis=1))
        wo = np.ascontiguousarray(
            w_o[c * WO_K:(c + 1) * WO_K, :]).astype(ml_dtypes.bfloat16)
        in_maps.append({"hT": hT, "wq": wq, "wo": wo, "cs": cs, "sn": sn,
                        "mk": mk, "on": ones})
    return in_maps


def run(positions, hidden_states, w_qkv, w_o, trace=False):
    from concourse import bass_utils
    nc = _get_program()
    in_maps = _host_inputs(positions, hidden_states, w_qkv, w_o)
    res = bass_utils.run_bass_kernel_spmd(
        nc, in_maps, core_ids=list(range(N_CORES)), trace=trace)
    acc = np.zeros((H, TOK), dtype=np.float32)
    for c in range(N_CORES):
        acc += res.results[c]["outT"]
    out = np.ascontiguousarray(acc.T).reshape(B, S, H)
    return out, res


def kernel(positions, hidden_states, w_qkv, w_o):
    out, _ = run(positions, hidden_states, w_qkv, w_o, trace=False)
    return out
